# revision 13
# baseline (speedup 1.0000x reference)
"""2-layer GCN encoder (GCNConv -> BN -> ReLU, x2) on 8 Trainium2 cores.

Sharding: nodes are assigned to 8*49 windows of 128 targets by a host-side
LPT bin-packing on in-degree, so every (core, window) has <= CW*128 incoming
edges (CW=17 for this graph's stats). Weights are folded into the gathered
table: each layer all-gathers (dinv * h) @ W in bf16, so the one-hot
scatter-matmul aggregation directly yields the conv output, target-major.

Per window: CW indirect DMAs gather 128 source rows each (the HW SWDGE ucode
honors one index per partition per call, so gathers cannot be batched wider),
DVE builds the one-hot sel via is_equal against an iota, and TensorE
accumulates sel^T @ rows straight into PSUM — target-major, so no transposes
or per-window weight matmuls in the hot loop. BN absorbs the conv bias.
"""

import sys

sys.path.insert(0, "/opt/trn_rl_repo")

import numpy as np
import ml_dtypes

import concourse.tile as tile
from concourse import bass, bacc, mybir
from concourse.bass_utils import run_bass_kernel_spmd

N = 50000
E = 800000
D = 256
NCORES = 8
P = 128
NW = 49                      # windows (of 128 targets) per core
SHARD = NW * P               # 6272 padded rows per core
NT = NCORES * SHARD          # 50176 padded table rows
EPS = 1e-5
PAD_DST = 300.0              # sentinel; never equals iota 0..127

f32 = mybir.dt.float32
bf16 = mybir.dt.bfloat16
i32 = mybir.dt.int32
bf = ml_dtypes.bfloat16


def _host_plan(edge_index):
    """Integer-only host preprocessing.

    Returns (CW, perm, pos, src_maps, dst_maps, dinv_maps) where perm[k*SHARD
    + w*128 + p] is the node at (core k, window w, row p) or -1 for padding,
    and pos[node] is its padded table row id.
    """
    row = np.asarray(edge_index[0], dtype=np.int64)
    col = np.asarray(edge_index[1], dtype=np.int64)
    deg = np.bincount(col, minlength=N).astype(np.int64) + 1  # + self-loop

    # LPT: nodes in degree-desc order into 392 bins (cap 128 nodes each),
    # always into the currently lightest bin -> near-perfect edge balance.
    import heapq
    nbins = NCORES * NW
    order = np.argsort(-deg, kind="stable")
    heap = [(0, b) for b in range(nbins)]
    heapq.heapify(heap)
    bin_nodes = [[] for _ in range(nbins)]
    for n in order:
        while True:
            s, b = heapq.heappop(heap)
            if len(bin_nodes[b]) < P:
                break
        bin_nodes[b].append(n)
        if len(bin_nodes[b]) < P:
            heapq.heappush(heap, (s + int(deg[n]), b))
    max_edges = max(int(deg[np.array(bn, dtype=np.int64)].sum()) if bn else 0
                    for bn in bin_nodes)
    CW = max((max_edges + P - 1) // P, 1)

    perm = np.full(NT, -1, dtype=np.int64)
    pos = np.empty(N, dtype=np.int64)
    for b, bn in enumerate(bin_nodes):
        a = np.asarray(bn, dtype=np.int64)
        perm[b * P:b * P + len(a)] = a
        pos[a] = b * P + np.arange(len(a), dtype=np.int64)

    # Edges sorted by (bin of target, arbitrary): slot stream per bin.
    tpos = pos[col]                      # padded row of each edge's target
    tbin = tpos // P
    eorder = np.argsort(tbin, kind="stable")
    src_sorted = pos[row[eorder]].astype(np.int32)   # table row of source
    dst_sorted = (tpos[eorder] % P).astype(np.float32)
    bin_of_edge = tbin[eorder]
    bounds = np.searchsorted(bin_of_edge, np.arange(nbins + 1))

    dinv = 1.0 / np.sqrt(deg.astype(np.float64))

    src_maps, dst_maps, dinv_maps = [], [], []
    for k in range(NCORES):
        src_k = np.zeros((NW, CW * P), dtype=np.int32)
        dst_k = np.full((NW, CW * P), PAD_DST, dtype=np.float32)
        for w in range(NW):
            b = k * NW + w
            lo, hi = int(bounds[b]), int(bounds[b + 1])
            ne = hi - lo                         # edges incl. implicit loops?
            # insert the bin's edges
            src_k[w, :ne] = src_sorted[lo:hi]
            dst_k[w, :ne] = dst_sorted[lo:hi]
            # self-loops for the bin's real nodes
            bn = perm[b * P:(b + 1) * P]
            real = np.nonzero(bn >= 0)[0]
            assert ne + len(real) <= CW * P
            src_k[w, ne:ne + len(real)] = (b * P + real).astype(np.int32)
            dst_k[w, ne:ne + len(real)] = real.astype(np.float32)
        # slot j -> (chunk c=j//P, part p=j%P); device layout [P, NW*CW]
        src_maps.append(np.ascontiguousarray(src_k.reshape(NW * CW, P).T))
        dst_maps.append(np.ascontiguousarray(
            dst_k.reshape(NW * CW, P).T.astype(bf)))
        dv = np.ones((NW, P), dtype=np.float64)
        bn = perm[k * SHARD:(k + 1) * SHARD].reshape(NW, P)
        m = bn >= 0
        dv[m] = dinv[bn[m]]
        dinv_maps.append(np.ascontiguousarray(dv.T.astype(np.float32)))

    iota = np.tile(np.arange(P, dtype=np.float32), (P, 1)).astype(bf)
    ident = np.eye(P, dtype=np.float32).astype(bf)
    return CW, perm, pos, src_maps, dst_maps, dinv_maps, iota, ident


def _build_program(CW):
    nc = bacc.Bacc("TRN2", target_bir_lowering=False, debug=False,
                   num_devices=NCORES)

    x_in = nc.dram_tensor("x_shard", [SHARD, D], f32, kind="ExternalInput")
    w_in = [nc.dram_tensor(f"W{l}", [D, D], bf16, kind="ExternalInput")
            for l in (1, 2)]
    g_in = [nc.dram_tensor(f"g{l}", [1, D], f32, kind="ExternalInput")
            for l in (1, 2)]
    be_in = [nc.dram_tensor(f"be{l}", [1, D], f32, kind="ExternalInput")
             for l in (1, 2)]
    dinv_in = nc.dram_tensor("dinv", [P, NW], f32, kind="ExternalInput")
    src_in = nc.dram_tensor("src", [P, NW * CW], i32, kind="ExternalInput")
    dst_in = nc.dram_tensor("dst", [P, NW * CW], bf16, kind="ExternalInput")
    iota_in = nc.dram_tensor("iota", [P, P], bf16, kind="ExternalInput")
    id_in = nc.dram_tensor("ident", [P, P], bf16, kind="ExternalInput")
    out_dram = nc.dram_tensor("out", [SHARD, D], f32, kind="ExternalOutput")

    # internal DRAM
    t_shard = [nc.dram_tensor(f"t{l}_shard", [SHARD, D], bf16) for l in (0, 1)]
    t_full = [nc.dram_tensor(f"t{l}_full", [NT, D], bf16, addr_space="Shared")
              for l in (0, 1)]
    sums_in = [nc.dram_tensor(f"sums_in{l}", [1, 2 * D], f32) for l in (1, 2)]
    sums_out = [nc.dram_tensor(f"sums_out{l}", [1, 2 * D], f32,
                               addr_space="Shared") for l in (1, 2)]

    rg = [list(range(NCORES))]
    NPAIR = (NW + 1) // 2

    with tile.TileContext(nc) as tc:
        import contextlib
        with contextlib.ExitStack() as ctx:
            const = ctx.enter_context(tc.tile_pool(name="const", bufs=1))
            work = ctx.enter_context(tc.tile_pool(name="work", bufs=4))
            gpool = ctx.enter_context(tc.tile_pool(name="gath", bufs=60))
            selp = ctx.enter_context(tc.tile_pool(name="sel", bufs=6))
            stashp = ctx.enter_context(tc.tile_pool(name="stash", bufs=NW + 2))
            vecp = ctx.enter_context(tc.tile_pool(name="vec", bufs=2))
            trp = ctx.enter_context(tc.tile_pool(name="trsb", bufs=4))
            psA = ctx.enter_context(tc.tile_pool(name="psA", bufs=3,
                                                 space="PSUM"))
            psT = ctx.enter_context(tc.tile_pool(name="psT", bufs=1,
                                                 space="PSUM"))
            psW = ctx.enter_context(tc.tile_pool(name="psW", bufs=2,
                                                 space="PSUM"))
            psC = ctx.enter_context(tc.tile_pool(name="psC", bufs=1,
                                                 space="PSUM"))

            # ---- constants ----
            iota_t = const.tile([P, P], bf16)
            nc.sync.dma_start(iota_t[:], iota_in[:])
            ident_t = const.tile([P, P], bf16)
            nc.sync.dma_start(ident_t[:], id_in[:])
            src_t = const.tile([P, NW * CW], i32)
            nc.sync.dma_start(src_t[:], src_in[:])
            dst_t = const.tile([P, NW * CW], bf16)
            nc.sync.dma_start(dst_t[:], dst_in[:])
            dinv_t = const.tile([P, NW], f32)
            nc.sync.dma_start(dinv_t[:], dinv_in[:])
            ones_col = const.tile([P, 1], bf16)
            nc.gpsimd.memset(ones_col[:], 1.0)
            ones_row = const.tile([1, P], f32)
            nc.gpsimd.memset(ones_row[:], 1.0)
            w_sb = []
            for l in range(2):
                halves = []
                for fh in range(2):
                    wt = const.tile([P, D], bf16, tag=f"w{l}{fh}")
                    nc.sync.dma_start(wt[:], w_in[l][fh * P:(fh + 1) * P, :])
                    halves.append(wt)
                w_sb.append(halves)
            gv, bev = [], []
            for l in range(2):
                t1 = const.tile([1, D], f32, tag=f"gv{l}")
                nc.sync.dma_start(t1[:], g_in[l][:])
                gv.append(t1)
                t2 = const.tile([1, D], f32, tag=f"bev{l}")
                nc.sync.dma_start(t2[:], be_in[l][:])
                bev.append(t2)

            def to_table(src_bf, w, l, dest):
                """src_bf [P,D] bf16 target-major -> dest rows w*128..: (src@W[l])"""
                trs = []
                for fh in range(2):
                    tp = psT.tile([P, P], bf16, space="PSUM", tag="tr")
                    nc.tensor.transpose(tp[:], src_bf[:, fh * P:(fh + 1) * P],
                                        ident_t[:])
                    ts = trp.tile([P, P], bf16, tag=f"trsb{fh}")
                    nc.scalar.activation(ts[:], tp[:],
                                         mybir.ActivationFunctionType.Copy)
                    trs.append(ts)
                hw_ps = psW.tile([P, D], f32, space="PSUM", tag="hw")
                for fh in range(2):
                    nc.tensor.matmul(out=hw_ps[:], lhsT=trs[fh][:],
                                     rhs=w_sb[l][fh][:],
                                     start=(fh == 0), stop=(fh == 1))
                hb = work.tile([P, D], bf16, tag="hb")
                nc.vector.tensor_copy(hb[:], hw_ps[:])
                nc.sync.dma_start(dest[w * P:(w + 1) * P, :], hb[:])

            # ---- prologue: own-shard (dinv*x) @ W1 -> t0_shard ----
            for w in range(NW):
                xt = work.tile([P, D], f32, tag="xt")
                nc.sync.dma_start(xt[:], x_in[w * P:(w + 1) * P, :])
                xst = work.tile([P, D], bf16, tag="xst")
                nc.vector.tensor_scalar_mul(xst[:], xt[:], dinv_t[:, w:w + 1])
                to_table(xst, w, 0, t_shard[0])
            nc.gpsimd.collective_compute(
                "AllGather", mybir.AluOpType.bypass, replica_groups=rg,
                ins=[t_shard[0][:]], outs=[t_full[0][:]],
            )

            def layer(l, table, out_is_final):
                sum_tile = psC.tile([1, D], f32, space="PSUM", tag="sum",
                                    name=f"sum_{l}")
                ssq_tile = psC.tile([1, D], f32, space="PSUM", tag="ssq",
                                    name=f"ssq_{l}")
                sum_ps = sum_tile[:]
                ssq_ps = ssq_tile[:]
                stash = []
                for pr in range(NPAIR):
                    w0 = 2 * pr
                    nwin = min(2, NW - w0)
                    cwp = nwin * CW
                    off = w0 * CW
                    gts = []
                    for cc in range(cwp):
                        gt = gpool.tile([P, D], bf16, tag="gt")
                        nc.gpsimd.indirect_dma_start(
                            out=gt[:], out_offset=None,
                            in_=table[:],
                            in_offset=bass.IndirectOffsetOnAxis(
                                ap=src_t[:, off + cc:off + cc + 1], axis=0),
                        )
                        gts.append(gt)
                    sel_t = selp.tile([P, cwp * P], bf16, tag="sel")
                    nc.vector.tensor_tensor(
                        out=sel_t[:].rearrange("p (k t) -> p k t", k=cwp),
                        in0=dst_t[:, off:off + cwp].unsqueeze(2)
                            .to_broadcast([P, cwp, P]),
                        in1=iota_t[:].unsqueeze(1).to_broadcast([P, cwp, P]),
                        op=mybir.AluOpType.is_equal,
                    )
                    for wi in range(nwin):
                        w = w0 + wi
                        agg = psA.tile([P, D], f32, space="PSUM", tag="agg")
                        for c in range(CW):
                            cc = wi * CW + c
                            nc.tensor.matmul(
                                out=agg[:],
                                lhsT=sel_t[:, cc * P:(cc + 1) * P],
                                rhs=gts[cc][:],
                                start=(c == 0), stop=(c == CW - 1),
                            )
                        st = stashp.tile([P, D], bf16, tag="stash")
                        nc.vector.tensor_scalar_mul(st[:], agg[:],
                                                    dinv_t[:, w:w + 1])
                        stash.append(st)
                        sq = work.tile([P, D], bf16, tag="sq")
                        nc.vector.tensor_mul(sq[:], st[:], st[:])
                        nc.tensor.matmul(out=sum_ps, lhsT=ones_col[:],
                                         rhs=st[:], start=(w == 0),
                                         stop=(w == NW - 1))
                        nc.tensor.matmul(out=ssq_ps, lhsT=ones_col[:],
                                         rhs=sq[:], start=(w == 0),
                                         stop=(w == NW - 1))

                # stats finalize + tiny all-reduce
                sums_sb = vecp.tile([1, 2 * D], f32, tag="sums")
                nc.vector.tensor_copy(sums_sb[:, :D], sum_ps)
                nc.vector.tensor_copy(sums_sb[:, D:], ssq_ps)
                nc.sync.dma_start(sums_in[l][:], sums_sb[:])
                nc.gpsimd.collective_compute(
                    "AllReduce", mybir.AluOpType.add, replica_groups=rg,
                    ins=[sums_in[l][:]], outs=[sums_out[l][:]],
                )
                gs = vecp.tile([1, 2 * D], f32, tag="gs")
                nc.sync.dma_start(gs[:], sums_out[l][:])
                mvec = vecp.tile([1, D], f32, tag="mvec")
                nc.vector.tensor_scalar_mul(mvec[:], gs[:, :D], 1.0 / N)
                evec = vecp.tile([1, D], f32, tag="evec")
                nc.vector.tensor_scalar_mul(evec[:], gs[:, D:], 1.0 / N)
                msq = vecp.tile([1, D], f32, tag="msq")
                nc.vector.tensor_mul(msq[:], mvec[:], mvec[:])
                var = vecp.tile([1, D], f32, tag="var")
                nc.vector.tensor_sub(var[:], evec[:], msq[:])
                vare = vecp.tile([1, D], f32, tag="vare")
                nc.vector.tensor_scalar_add(vare[:], var[:], EPS)
                sdv = vecp.tile([1, D], f32, tag="sdv")
                nc.scalar.activation(sdv[:], vare[:],
                                     mybir.ActivationFunctionType.Sqrt)
                rsv = vecp.tile([1, D], f32, tag="rsv")
                nc.vector.reciprocal(rsv[:], sdv[:])
                bc_in = vecp.tile([1, 2 * D], f32, tag="bcin")
                nc.vector.tensor_mul(bc_in[:, :D], rsv[:], gv[l][:])  # scale
                sh1 = vecp.tile([1, D], f32, tag="sh1")
                nc.vector.tensor_mul(sh1[:], mvec[:], bc_in[:, :D])
                nc.vector.tensor_sub(bc_in[:, D:], bev[l][:], sh1[:])  # shift
                bc_ps = psW.tile([P, 2 * D], f32, space="PSUM", tag="hw",
                                 name=f"bc_ps_{l}")
                nc.tensor.matmul(out=bc_ps[:], lhsT=ones_row[:], rhs=bc_in[:],
                                 start=True, stop=True)
                bc_sb = work.tile([P, 2 * D], f32, tag="bcsb")
                nc.vector.tensor_copy(bc_sb[:], bc_ps[:])

                # normalize + relu (+ next-layer W fold) + store
                for w in range(NW):
                    t1 = work.tile([P, D], f32, tag="t1")
                    nc.vector.tensor_mul(t1[:], stash[w][:], bc_sb[:, :D])
                    t2 = work.tile([P, D], f32, tag="t2")
                    nc.vector.tensor_add(t2[:], t1[:], bc_sb[:, D:])
                    if out_is_final:
                        ot = work.tile([P, D], f32, tag="ot")
                        nc.vector.tensor_scalar_max(ot[:], t2[:], 0.0)
                        nc.sync.dma_start(out_dram[w * P:(w + 1) * P, :],
                                          ot[:])
                    else:
                        ht = work.tile([P, D], bf16, tag="ht")
                        nc.vector.tensor_scalar(
                            out=ht[:], in0=t2[:],
                            scalar1=0.0, scalar2=dinv_t[:, w:w + 1],
                            op0=mybir.AluOpType.max,
                            op1=mybir.AluOpType.mult,
                        )
                        to_table(ht, w, 1, t_shard[1])

            layer(0, t_full[0], False)
            nc.gpsimd.collective_compute(
                "AllGather", mybir.AluOpType.bypass, replica_groups=rg,
                ins=[t_shard[1][:]], outs=[t_full[1][:]],
            )
            layer(1, t_full[1], True)

    nc.compile()
    return nc


_prog_cache = {}


def kernel(**inputs):
    x = np.asarray(inputs["x"], dtype=np.float32)
    edge_index = np.asarray(inputs["edge_index"])
    W = [np.asarray(inputs["W1"], np.float32),
         np.asarray(inputs["W2"], np.float32)]
    g = [np.asarray(inputs["g1"], np.float32),
         np.asarray(inputs["g2"], np.float32)]
    be = [np.asarray(inputs["be1"], np.float32),
          np.asarray(inputs["be2"], np.float32)]

    (CW, perm, pos, src_maps, dst_maps, dinv_maps,
     iota, ident) = _host_plan(edge_index)
    if CW not in _prog_cache:
        _prog_cache[CW] = _build_program(CW)
    nc = _prog_cache[CW]

    xp = np.zeros((NT, D), dtype=np.float32)
    real = perm >= 0
    xp[real] = x[perm[real]]

    in_maps = []
    for k in range(NCORES):
        in_maps.append({
            "x_shard": np.ascontiguousarray(xp[k * SHARD:(k + 1) * SHARD]),
            "W1": W[0].astype(bf), "W2": W[1].astype(bf),
            "g1": g[0].reshape(1, D), "g2": g[1].reshape(1, D),
            "be1": be[0].reshape(1, D), "be2": be[1].reshape(1, D),
            "dinv": dinv_maps[k],
            "src": src_maps[k], "dst": dst_maps[k],
            "iota": iota, "ident": ident,
        })
    res = run_bass_kernel_spmd(nc, in_maps, list(range(NCORES)),
                               **inputs.get("_run_kwargs", {}))
    outp = np.concatenate([res.results[k]["out"] for k in range(NCORES)],
                          axis=0)
    out = np.empty((N, D), dtype=np.float32)
    out[perm[real]] = outp[real]
    if "_res_sink" in inputs:
        inputs["_res_sink"].append(res)
    return out
